# revision 32
# baseline (speedup 1.0000x reference)
"""LocallyConnected2d kernel for 8 TRN2 NeuronCores (Bass/Tile).

Problem (hardcoded):
  features [32, 64, 64, 64] f32, weights [62, 62, 64, 64, 3, 3] f32,
  bias [62, 62, 64] f32 -> out [32, 64, 62, 62] f32
  out[b,o,h,w] = sum_{c,i,j} x[b,c,h+i,w+j] * W[h,w,o,c,i,j] + bias[h,w,o]

Strategy (v9 - full 8-row band per work unit):
  - Shard over Hout: 8 cores x 8 output rows (bands [0,8,...,48,54], last two
    overlap; host takes canonical rows from each core).
  - Weights stream as fp8 e3m4 (x2 scale, /2 on host) = 1 B/el -> 18.9 MB/core
    with zero padding waste; activations stay bf16 (mixed-dtype matmul: only
    fp32 operands must be paired). PSUM accumulates fp32. rel err ~0.014.
  - fA layout [128=(c | c shifted w+1), w, t, b]; a [128,32] slice at (w,t) is
    an im2col patch: lower half = x(w), upper = x(w+1).
  - Work unit = one wg (4 w positions), ALL 8 output rows at once: PSUM
    [128, 512] (one full bank): partitions=(4w x 32b) via col tile_position,
    free=(8j x 64 cout). tau = t in 0..9, valid j in [max(0,t-2), min(7,t)].
    * PSUM zeroed OFF the PE (ACT copy-from-zeros / DVE memset); matmuls all
      run flags=0 (has_written: accumulate onto the zeros where stale-set,
      overwrite the zeros where clear). No start=True anywhere.
    * wr MMs (taps i in {0,1}): stationary fA[:, w0+g, tau] K=128, moving
      N=nv*64 <= 192. 40 MMs/wg.
    * tap i=2 is K=64, tau-grouped, split across TWO psum tiles because one
      accumulation group must not mix PE row-tiles (HW limitation, probed):
      taus {0,1,2,5,8,9} via fA lower (w0+g+2) -> main ps (row tile 0);
      taus {3,4,6,7} via the w+1-shifted upper half (w0+g+1, same x column)
      -> psB (row tile 64). psB's j0 columns are never written and stay
      memset-zero. Both wk64 partition halves fully used -> no padding.
    * combine: ACT copies psB -> S slice (bf16), DVE tensor_add(S, ps, S).
  - DMA: 16 wk transfers [128, 9216] fp8 (1.18 MB) on sync; fA w-chunks +
    eighth-granularity outS dumps on scalar.
"""

import numpy as np
import ml_dtypes

BF16 = ml_dtypes.bfloat16
F8E3 = ml_dtypes.float8_e3m4
WSCALE = np.float32(2.0)

B, CIN, COUT = 32, 64, 64
H = W = 64
HOUT = WOUT = 62
NCORES = 8
STARTS = [0, 8, 16, 24, 32, 40, 48, 54]

# tau-group geometry over the full 8-row band: tau = t in 0..9
TAUS = list(range(10))
JLO = [max(0, t - 2) for t in TAUS]
JHI = [min(7, t) for t in TAUS]
NV = [hi - lo + 1 for lo, hi in zip(JLO, JHI)]    # [1,2,3,3,3,3,3,3,2,1]
TBASE = [0]
for t in TAUS:
    TBASE.append(TBASE[-1] + 4 * NV[t] * 64)
WR_COLS = TBASE[-1]                                # 6144
# K64 split: 12 nv-units each side
K64_LOW_TAUS = [0, 1, 2, 5, 8, 9]                  # nv 1+2+3+3+2+1 = 12
K64_HIGH_TAUS = [3, 4, 6, 7]                       # nv 3+3+3+3 = 12
K64LO = {}
off = 0
for t in K64_LOW_TAUS:
    K64LO[t] = off
    off += 4 * NV[t] * 64
K64HI = {}
off = 0
for t in K64_HIGH_TAUS:
    K64HI[t] = off
    off += 4 * NV[t] * 64
K64_COLS = 3072                                    # both halves, 12 units
WG_COLS = WR_COLS + K64_COLS                       # 9216
# wg15 keeps only g in {2,3} (slots 60-61 duplicate w58-59): compact block
TBASE15 = [0]
for t in TAUS:
    TBASE15.append(TBASE15[-1] + 2 * NV[t] * 64)   # 3072
WR15_COLS = TBASE15[-1]
K64LO15 = {}
_o = 0
for t in K64_LOW_TAUS:
    K64LO15[t] = _o
    _o += 2 * NV[t] * 64
K64HI15 = {}
_o = 0
for t in K64_HIGH_TAUS:
    K64HI15[t] = _o
    _o += 2 * NV[t] * 64
WG15_COLS = WR15_COLS + 1536                       # 4608

_STATE = {}


def _build_program():
    import concourse.tile as tile
    from concourse import bacc, mybir

    bf = mybir.dt.bfloat16
    f8 = mybir.dt.float8e3
    f32 = mybir.dt.float32
    ACT_COPY = mybir.ActivationFunctionType.Copy

    nc = bacc.Bacc(None, target_bir_lowering=False)
    featA = nc.dram_tensor("featA", [128, 64, 10, 32], bf, kind="ExternalInput")
    wk_d = nc.dram_tensor("wk", [16, 128, WG_COLS], f8, kind="ExternalInput")
    outS = nc.dram_tensor("outS", [128, 8192], bf, kind="ExternalOutput")

    with tile.TileContext(nc) as tc:
        with tc.tile_pool(name="feat", bufs=1) as fpool, \
             tc.tile_pool(name="wk", bufs=8) as wkpool, \
             tc.tile_pool(name="st", bufs=1) as spool, \
             tc.tile_pool(name="ps", bufs=5, space="PSUM") as pspool, \
             tc.tile_pool(name="psb", bufs=3, space="PSUM") as psbpool:
            fA = fpool.tile([128, 64, 10, 32], bf)
            nc.scalar.dma_start(fA[:, 0:8], featA[:, 0:8])
            nc.scalar.dma_start(fA[:, 8:16], featA[:, 8:16])
            nc.scalar.dma_start(fA[:, 16:24], featA[:, 16:24])
            nc.scalar.dma_start(fA[:, 24:32], featA[:, 24:32])
            nc.scalar.dma_start(fA[:, 32:48], featA[:, 32:48])
            nc.scalar.dma_start(fA[:, 48:64], featA[:, 48:64])
            # zeros for the ACT psum-clearing copies
            zb = fpool.tile([128, 512], bf)
            nc.gpsimd.memset(zb[:], 0.0)
            S = spool.tile([128, 8192], bf)
            for wg in range(16):
                wk = wkpool.tile([128, WG_COLS], f8)
                if wg <= 2:
                    # split the ramp-phase transfers so each wg's wr slice
                    # (consumed first) lands sooner while DMA is slowest
                    nc.sync.dma_start(wk[:, 0:WR_COLS],
                                      wk_d[wg][:, 0:WR_COLS])
                    nc.sync.dma_start(wk[:, WR_COLS:WG_COLS],
                                      wk_d[wg][:, WR_COLS:WG_COLS])
                elif wg == 15:
                    # compact block: only g in {2,3} (slots 60-61 are dups)
                    nc.sync.dma_start(wk[:, 0:WG15_COLS],
                                      wk_d[15][:, 0:WG15_COLS])
                else:
                    nc.sync.dma_start(wk[:], wk_d[wg])
                w0 = min(4 * wg, 58)      # last group overlaps: w 58..61
                if wg == 15:
                    wr = wk[:, 0:WR15_COLS]
                    wk64 = wk[:, WR15_COLS:WG15_COLS]
                    glist, gb, tb, klo, khi = [2, 3], 2, TBASE15, K64LO15, K64HI15
                else:
                    wr = wk[:, 0:WR_COLS]
                    wk64 = wk[:, WR_COLS:WG_COLS]
                    glist, gb, tb, klo, khi = [0, 1, 2, 3], 0, TBASE, K64LO, K64HI

                ps = pspool.tile([128, 512], f32)
                psB = psbpool.tile([128, 512], f32)
                # Zero PSUM off the PE (see docstring).
                nc.scalar.activation(ps[:, :], zb[:, :], ACT_COPY)
                nc.vector.memset(psB[:, :], 0.0)
                # taps i in {0,1}: K=128 dual-w stationaries
                for tau in TAUS:
                    nv, jlo = NV[tau], JLO[tau]
                    for g in glist:
                        off = tb[tau] + (g - gb) * nv * 64
                        nc.tensor.matmul(
                            ps[32 * g:32 * g + 32,
                               64 * jlo:64 * (jlo + nv)],
                            fA[:, w0 + g, tau, :],
                            wr[:, off:off + nv * 64],
                            start=False, stop=False,
                            skip_group_check=True,
                            tile_position=(0, 32 * g),
                        )
                # tap i=2, taus {3,4,6,7}: K=64 upper halves -> psB first
                # (psB finishes early so its ACT copy overlaps later MMs)
                for ti, tau in enumerate(K64_HIGH_TAUS):
                    nv, jlo = NV[tau], JLO[tau]
                    for g in glist:
                        off = khi[tau] + (g - gb) * nv * 64
                        nc.tensor.matmul(
                            psB[32 * g:32 * g + 32,
                                64 * jlo:64 * (jlo + nv)],
                            fA[64:128, w0 + g + 1, tau, :],
                            wk64[64:128, off:off + nv * 64],
                            start=False,
                            stop=(ti == 3 and g == 3),
                            skip_group_check=True,
                            tile_position=(64, 32 * g),
                        )
                # tap i=2, taus {0,1,2,5,8,9}: K=64 lower halves -> main
                for ti, tau in enumerate(K64_LOW_TAUS):
                    nv, jlo = NV[tau], JLO[tau]
                    for g in glist:
                        off = klo[tau] + (g - gb) * nv * 64
                        nc.tensor.matmul(
                            ps[32 * g:32 * g + 32,
                               64 * jlo:64 * (jlo + nv)],
                            fA[0:64, w0 + g + 2, tau, :],
                            wk64[0:64, off:off + nv * 64],
                            start=False,
                            stop=(ti == 5 and g == 3),
                            skip_group_check=True,
                            tile_position=(0, 32 * g),
                        )
                sl = S[:, 512 * wg:512 * wg + 512]
                nc.scalar.activation(sl, psB[:, :], ACT_COPY)
                nc.vector.tensor_add(sl, ps[:], sl)
                if wg % 2 == 1:
                    # eighth-granularity dump keeps the tail short
                    nc.scalar.dma_start(
                        outS[:, 1024 * (wg // 2):1024 * (wg // 2 + 1)],
                        S[:, 1024 * (wg // 2):1024 * (wg // 2 + 1)])
    nc.compile()
    return nc


def _get_nc():
    if "nc" not in _STATE:
        _STATE["nc"] = _build_program()
    return _STATE["nc"]


def _quant_w(a):
    return np.clip(a * WSCALE, -15.0, 15.0).astype(F8E3)


def _prep_inputs(features, weights):
    """Build the 8 per-core input dicts (device layouts)."""
    x = np.asarray(features, dtype=np.float32)
    Wt = np.asarray(weights, dtype=np.float32)

    # w-slot -> real w: last group overlaps (w 58..61)
    widx = list(range(60)) + [58, 59, 60, 61]

    in_maps = []
    for s in STARTS:
        xt = x[:, :, s:s + 10, :].transpose(1, 3, 2, 0)    # [c, w, t, b]
        fA = np.zeros((128, 64, 10, 32), dtype=BF16)
        fA[:64] = xt
        fA[64:, 0:63] = xt[:, 1:]                          # w+1 shift

        Wb = Wt[s:s + 8]                                   # [8, 62, o, c, 3, 3]
        Wsel = Wb[:, widx]                                 # [8, 64slots, o, c, 3, 3]
        WT = Wsel.transpose(4, 5, 3, 0, 1, 2)              # [r, i, c, 8h(j), 64w, o]

        wkf = np.zeros((16, 128, WG_COLS), dtype=np.float32)
        # wr: taps (r, i=d); cols per (tau, g): q -> j=jlo+q, r=tau-j
        wr = wkf[:, :, 0:WR_COLS]
        for tau in TAUS:
            nv, jlo = NV[tau], JLO[tau]
            view = wr[:, :, TBASE[tau]:TBASE[tau + 1]].reshape(
                16, 128, 4, nv, 64)
            for q in range(nv):
                j = jlo + q
                r = tau - j
                for d in range(2):
                    src = WT[r, d][:, j].reshape(CIN, 16, 4, COUT)
                    view[:, d * 64:(d + 1) * 64, :, q, :] = \
                        src.transpose(1, 0, 2, 3)          # [wg, c, g, o]
        # wk64: tap i=2; low taus at partitions 0:64, high at 64:128
        wk64 = wkf[:, :, WR_COLS:WG_COLS]
        for tau in TAUS:
            nv, jlo = NV[tau], JLO[tau]
            if tau in K64LO:
                p0, cb = 0, K64LO[tau]
            else:
                p0, cb = 64, K64HI[tau]
            view = wk64[:, p0:p0 + 64, cb:cb + 4 * nv * 64].reshape(
                16, 64, 4, nv, 64)
            for q in range(nv):
                j = jlo + q
                r = tau - j
                src = WT[r, 2][:, j].reshape(CIN, 16, 4, COUT)
                view[:, :, :, q, :] = src.transpose(1, 0, 2, 3)
        # repack wg15: keep only g in {2,3}, compact layout
        blk = wkf[15]
        blk15 = np.zeros((128, WG_COLS), dtype=np.float32)
        for tau in TAUS:
            nv = NV[tau]
            v = blk[:, TBASE[tau]:TBASE[tau + 1]].reshape(128, 4, nv, 64)
            blk15[:, TBASE15[tau]:TBASE15[tau + 1]] = \
                v[:, 2:4].reshape(128, 2 * nv * 64)
        for tau in TAUS:
            nv = NV[tau]
            if tau in K64LO:
                p0, cb, cb15 = 0, K64LO[tau], K64LO15[tau]
            else:
                p0, cb, cb15 = 64, K64HI[tau], K64HI15[tau]
            v = blk[p0:p0 + 64, WR_COLS + cb:WR_COLS + cb + 4 * nv * 64]
            v = v.reshape(64, 4, nv, 64)
            blk15[p0:p0 + 64, WR15_COLS + cb15:WR15_COLS + cb15 + 2 * nv * 64] = \
                v[:, 2:4].reshape(64, 2 * nv * 64)
        wkf[15] = blk15
        wk = np.ascontiguousarray(_quant_w(wkf))
        in_maps.append({"featA": fA, "wk": wk})
    return in_maps


def _gather(results, bias):
    out = np.zeros((B, COUT, HOUT, WOUT), dtype=np.float32)
    inv = 1.0 / float(WSCALE)
    for core, s in enumerate(STARTS):
        arr = np.asarray(results[core]["outS"]).astype(np.float32) * inv
        # [g, b, wg, j, o] -> [b, o, j, wg, g]
        arr = arr.reshape(4, 32, 16, 8, 64).transpose(1, 4, 3, 2, 0)
        arr = arr.reshape(32, 64, 8, 64)
        out[:, :, s:s + 8, 0:60] = arr[:, :, :, 0:60]
        out[:, :, s:s + 8, 60:62] = arr[:, :, :, 62:64]
    out += np.asarray(bias, dtype=np.float32).transpose(2, 0, 1)[None]
    return out


def _run(in_maps, trace=False, trace_cores=None):
    from concourse.bass_utils import run_bass_kernel_spmd
    nc = _get_nc()
    return run_bass_kernel_spmd(
        nc, in_maps, core_ids=list(range(NCORES)),
        trace=trace, trace_cores=trace_cores,
    )


def kernel(features, weights, bias):
    in_maps = _prep_inputs(features, weights)
    res = _run(in_maps)
    return _gather(res.results, bias)


# revision 33
# speedup vs baseline: 1.1057x; 1.1057x over previous
"""LocallyConnected2d kernel for 8 TRN2 NeuronCores (Bass/Tile).

Problem (hardcoded):
  features [32, 64, 64, 64] f32, weights [62, 62, 64, 64, 3, 3] f32,
  bias [62, 62, 64] f32 -> out [32, 64, 62, 62] f32
  out[b,o,h,w] = sum_{c,i,j} x[b,c,h+i,w+j] * W[h,w,o,c,i,j] + bias[h,w,o]

Strategy (v9 - full 8-row band per work unit):
  - Shard over Hout: 8 cores x 8 output rows (bands [0,8,...,48,54], last two
    overlap; host takes canonical rows from each core).
  - Weights stream as fp8 e3m4 (x2 scale, /2 on host) = 1 B/el -> 18.9 MB/core
    with zero padding waste; activations stay bf16 (mixed-dtype matmul: only
    fp32 operands must be paired). PSUM accumulates fp32. rel err ~0.014.
  - fA layout [128=(c | c shifted w+1), w, t, b]; a [128,32] slice at (w,t) is
    an im2col patch: lower half = x(w), upper = x(w+1).
  - Work unit = one wg (4 w positions), ALL 8 output rows at once: PSUM
    [128, 512] (one full bank): partitions=(4w x 32b) via col tile_position,
    free=(8j x 64 cout). tau = t in 0..9, valid j in [max(0,t-2), min(7,t)].
    * PSUM zeroed OFF the PE (ACT copy-from-zeros / DVE memset); matmuls all
      run flags=0 (has_written: accumulate onto the zeros where stale-set,
      overwrite the zeros where clear). No start=True anywhere.
    * wr MMs (taps i in {0,1}): stationary fA[:, w0+g, tau] K=128, moving
      N=nv*64 <= 192. 40 MMs/wg.
    * tap i=2 is K=64, tau-grouped, split across TWO psum tiles because one
      accumulation group must not mix PE row-tiles (HW limitation, probed):
      taus {0,1,2,5,8,9} via fA lower (w0+g+2) -> main ps (row tile 0);
      taus {3,4,6,7} via the w+1-shifted upper half (w0+g+1, same x column)
      -> psB (row tile 64). psB's j0 columns are never written and stay
      memset-zero. Both wk64 partition halves fully used -> no padding.
    * combine: ACT copies psB -> S slice (bf16), DVE tensor_add(S, ps, S).
  - DMA: 16 wk transfers [128, 9216] fp8 (1.18 MB) on sync; fA w-chunks +
    eighth-granularity outS dumps on scalar.
"""

import numpy as np
import ml_dtypes

BF16 = ml_dtypes.bfloat16
F8E3 = ml_dtypes.float8_e3m4
WSCALE = np.float32(2.0)

B, CIN, COUT = 32, 64, 64
H = W = 64
HOUT = WOUT = 62
NCORES = 8
STARTS = [0, 8, 16, 24, 32, 40, 48, 54]

# tau-group geometry over the full 8-row band: tau = t in 0..9
TAUS = list(range(10))
JLO = [max(0, t - 2) for t in TAUS]
JHI = [min(7, t) for t in TAUS]
NV = [hi - lo + 1 for lo, hi in zip(JLO, JHI)]    # [1,2,3,3,3,3,3,3,2,1]
TBASE = [0]
for t in TAUS:
    TBASE.append(TBASE[-1] + 4 * NV[t] * 64)
WR_COLS = TBASE[-1]                                # 6144
# K64 split: 12 nv-units each side
K64_LOW_TAUS = [0, 1, 2, 5, 8, 9]                  # nv 1+2+3+3+2+1 = 12
K64_HIGH_TAUS = [3, 4, 6, 7]                       # nv 3+3+3+3 = 12
K64LO = {}
off = 0
for t in K64_LOW_TAUS:
    K64LO[t] = off
    off += 4 * NV[t] * 64
K64HI = {}
off = 0
for t in K64_HIGH_TAUS:
    K64HI[t] = off
    off += 4 * NV[t] * 64
K64_COLS = 3072                                    # both halves, 12 units
WG_COLS = WR_COLS + K64_COLS                       # 9216
# wg15 keeps only g in {2,3} (slots 60-61 duplicate w58-59): compact block
TBASE15 = [0]
for t in TAUS:
    TBASE15.append(TBASE15[-1] + 2 * NV[t] * 64)   # 3072
WR15_COLS = TBASE15[-1]
K64LO15 = {}
_o = 0
for t in K64_LOW_TAUS:
    K64LO15[t] = _o
    _o += 2 * NV[t] * 64
K64HI15 = {}
_o = 0
for t in K64_HIGH_TAUS:
    K64HI15[t] = _o
    _o += 2 * NV[t] * 64
WG15_COLS = WR15_COLS + 1536                       # 4608

_STATE = {}


def _build_program():
    import concourse.tile as tile
    from concourse import bacc, mybir

    bf = mybir.dt.bfloat16
    f8 = mybir.dt.float8e3
    f32 = mybir.dt.float32
    ACT_COPY = mybir.ActivationFunctionType.Copy

    nc = bacc.Bacc(None, target_bir_lowering=False)
    featA = nc.dram_tensor("featA", [128, 64, 10, 32], bf, kind="ExternalInput")
    wk_d = nc.dram_tensor("wk", [16, 128, WG_COLS], f8, kind="ExternalInput")
    outS = nc.dram_tensor("outS", [128, 8192], bf, kind="ExternalOutput")

    with tile.TileContext(nc) as tc:
        with tc.tile_pool(name="feat", bufs=1) as fpool, \
             tc.tile_pool(name="wk", bufs=8) as wkpool, \
             tc.tile_pool(name="st", bufs=1) as spool, \
             tc.tile_pool(name="ps", bufs=5, space="PSUM") as pspool, \
             tc.tile_pool(name="psb", bufs=3, space="PSUM") as psbpool:
            fA = fpool.tile([128, 64, 10, 32], bf)
            nc.scalar.dma_start(fA[:, 0:8], featA[:, 0:8])
            nc.scalar.dma_start(fA[:, 8:16], featA[:, 8:16])
            nc.scalar.dma_start(fA[:, 16:24], featA[:, 16:24])
            nc.scalar.dma_start(fA[:, 24:32], featA[:, 24:32])
            nc.scalar.dma_start(fA[:, 32:48], featA[:, 32:48])
            nc.scalar.dma_start(fA[:, 48:64], featA[:, 48:64])
            # zeros for the ACT psum-clearing copies
            zb = fpool.tile([128, 512], bf)
            nc.gpsimd.memset(zb[:], 0.0)
            S = spool.tile([128, 8192], bf)
            for wg in range(16):
                wk = wkpool.tile([128, WG_COLS], f8)
                if wg == 0:
                    # split so the wr slice (consumed first) lands sooner
                    nc.sync.dma_start(wk[:, 0:WR_COLS], wk_d[0][:, 0:WR_COLS])
                    nc.sync.dma_start(wk[:, WR_COLS:WG_COLS],
                                      wk_d[0][:, WR_COLS:WG_COLS])
                elif wg == 15:
                    # compact block: only g in {2,3} (slots 60-61 are dups)
                    nc.sync.dma_start(wk[:, 0:WG15_COLS],
                                      wk_d[15][:, 0:WG15_COLS])
                else:
                    nc.sync.dma_start(wk[:], wk_d[wg])
                w0 = min(4 * wg, 58)      # last group overlaps: w 58..61
                if wg == 15:
                    wr = wk[:, 0:WR15_COLS]
                    wk64 = wk[:, WR15_COLS:WG15_COLS]
                    glist, gb, tb, klo, khi = [2, 3], 2, TBASE15, K64LO15, K64HI15
                else:
                    wr = wk[:, 0:WR_COLS]
                    wk64 = wk[:, WR_COLS:WG_COLS]
                    glist, gb, tb, klo, khi = [0, 1, 2, 3], 0, TBASE, K64LO, K64HI

                ps = pspool.tile([128, 512], f32)
                psB = psbpool.tile([128, 512], f32)
                # Zero PSUM off the PE (see docstring).
                nc.scalar.activation(ps[:, :], zb[:, :], ACT_COPY)
                nc.vector.memset(psB[:, :], 0.0)
                # taps i in {0,1}: K=128 dual-w stationaries
                for tau in TAUS:
                    nv, jlo = NV[tau], JLO[tau]
                    for g in glist:
                        off = tb[tau] + (g - gb) * nv * 64
                        nc.tensor.matmul(
                            ps[32 * g:32 * g + 32,
                               64 * jlo:64 * (jlo + nv)],
                            fA[:, w0 + g, tau, :],
                            wr[:, off:off + nv * 64],
                            start=False, stop=False,
                            skip_group_check=True,
                            tile_position=(0, 32 * g),
                        )
                # tap i=2, taus {3,4,6,7}: K=64 upper halves -> psB first
                # (psB finishes early so its ACT copy overlaps later MMs)
                for ti, tau in enumerate(K64_HIGH_TAUS):
                    nv, jlo = NV[tau], JLO[tau]
                    for g in glist:
                        off = khi[tau] + (g - gb) * nv * 64
                        nc.tensor.matmul(
                            psB[32 * g:32 * g + 32,
                                64 * jlo:64 * (jlo + nv)],
                            fA[64:128, w0 + g + 1, tau, :],
                            wk64[64:128, off:off + nv * 64],
                            start=False,
                            stop=(ti == 3 and g == 3),
                            skip_group_check=True,
                            tile_position=(64, 32 * g),
                        )
                # tap i=2, taus {0,1,2,5,8,9}: K=64 lower halves -> main
                for ti, tau in enumerate(K64_LOW_TAUS):
                    nv, jlo = NV[tau], JLO[tau]
                    for g in glist:
                        off = klo[tau] + (g - gb) * nv * 64
                        nc.tensor.matmul(
                            ps[32 * g:32 * g + 32,
                               64 * jlo:64 * (jlo + nv)],
                            fA[0:64, w0 + g + 2, tau, :],
                            wk64[0:64, off:off + nv * 64],
                            start=False,
                            stop=(ti == 5 and g == 3),
                            skip_group_check=True,
                            tile_position=(0, 32 * g),
                        )
                sl = S[:, 512 * wg:512 * wg + 512]
                nc.scalar.activation(sl, psB[:, :], ACT_COPY)
                nc.vector.tensor_add(sl, ps[:], sl)
                if wg % 2 == 1:
                    # eighth-granularity dump keeps the tail short
                    nc.scalar.dma_start(
                        outS[:, 1024 * (wg // 2):1024 * (wg // 2 + 1)],
                        S[:, 1024 * (wg // 2):1024 * (wg // 2 + 1)])
    nc.compile()
    return nc


def _get_nc():
    if "nc" not in _STATE:
        _STATE["nc"] = _build_program()
    return _STATE["nc"]


def _quant_w(a):
    return np.clip(a * WSCALE, -15.0, 15.0).astype(F8E3)


def _prep_inputs(features, weights):
    """Build the 8 per-core input dicts (device layouts)."""
    x = np.asarray(features, dtype=np.float32)
    Wt = np.asarray(weights, dtype=np.float32)

    # w-slot -> real w: last group overlaps (w 58..61)
    widx = list(range(60)) + [58, 59, 60, 61]

    in_maps = []
    for s in STARTS:
        xt = x[:, :, s:s + 10, :].transpose(1, 3, 2, 0)    # [c, w, t, b]
        fA = np.zeros((128, 64, 10, 32), dtype=BF16)
        fA[:64] = xt
        fA[64:, 0:63] = xt[:, 1:]                          # w+1 shift

        Wb = Wt[s:s + 8]                                   # [8, 62, o, c, 3, 3]
        Wsel = Wb[:, widx]                                 # [8, 64slots, o, c, 3, 3]
        WT = Wsel.transpose(4, 5, 3, 0, 1, 2)              # [r, i, c, 8h(j), 64w, o]

        wkf = np.zeros((16, 128, WG_COLS), dtype=np.float32)
        # wr: taps (r, i=d); cols per (tau, g): q -> j=jlo+q, r=tau-j
        wr = wkf[:, :, 0:WR_COLS]
        for tau in TAUS:
            nv, jlo = NV[tau], JLO[tau]
            view = wr[:, :, TBASE[tau]:TBASE[tau + 1]].reshape(
                16, 128, 4, nv, 64)
            for q in range(nv):
                j = jlo + q
                r = tau - j
                for d in range(2):
                    src = WT[r, d][:, j].reshape(CIN, 16, 4, COUT)
                    view[:, d * 64:(d + 1) * 64, :, q, :] = \
                        src.transpose(1, 0, 2, 3)          # [wg, c, g, o]
        # wk64: tap i=2; low taus at partitions 0:64, high at 64:128
        wk64 = wkf[:, :, WR_COLS:WG_COLS]
        for tau in TAUS:
            nv, jlo = NV[tau], JLO[tau]
            if tau in K64LO:
                p0, cb = 0, K64LO[tau]
            else:
                p0, cb = 64, K64HI[tau]
            view = wk64[:, p0:p0 + 64, cb:cb + 4 * nv * 64].reshape(
                16, 64, 4, nv, 64)
            for q in range(nv):
                j = jlo + q
                r = tau - j
                src = WT[r, 2][:, j].reshape(CIN, 16, 4, COUT)
                view[:, :, :, q, :] = src.transpose(1, 0, 2, 3)
        # repack wg15: keep only g in {2,3}, compact layout
        blk = wkf[15]
        blk15 = np.zeros((128, WG_COLS), dtype=np.float32)
        for tau in TAUS:
            nv = NV[tau]
            v = blk[:, TBASE[tau]:TBASE[tau + 1]].reshape(128, 4, nv, 64)
            blk15[:, TBASE15[tau]:TBASE15[tau + 1]] = \
                v[:, 2:4].reshape(128, 2 * nv * 64)
        for tau in TAUS:
            nv = NV[tau]
            if tau in K64LO:
                p0, cb, cb15 = 0, K64LO[tau], K64LO15[tau]
            else:
                p0, cb, cb15 = 64, K64HI[tau], K64HI15[tau]
            v = blk[p0:p0 + 64, WR_COLS + cb:WR_COLS + cb + 4 * nv * 64]
            v = v.reshape(64, 4, nv, 64)
            blk15[p0:p0 + 64, WR15_COLS + cb15:WR15_COLS + cb15 + 2 * nv * 64] = \
                v[:, 2:4].reshape(64, 2 * nv * 64)
        wkf[15] = blk15
        wk = np.ascontiguousarray(_quant_w(wkf))
        in_maps.append({"featA": fA, "wk": wk})
    return in_maps


def _gather(results, bias):
    out = np.zeros((B, COUT, HOUT, WOUT), dtype=np.float32)
    inv = 1.0 / float(WSCALE)
    for core, s in enumerate(STARTS):
        arr = np.asarray(results[core]["outS"]).astype(np.float32) * inv
        # [g, b, wg, j, o] -> [b, o, j, wg, g]
        arr = arr.reshape(4, 32, 16, 8, 64).transpose(1, 4, 3, 2, 0)
        arr = arr.reshape(32, 64, 8, 64)
        out[:, :, s:s + 8, 0:60] = arr[:, :, :, 0:60]
        out[:, :, s:s + 8, 60:62] = arr[:, :, :, 62:64]
    out += np.asarray(bias, dtype=np.float32).transpose(2, 0, 1)[None]
    return out


def _run(in_maps, trace=False, trace_cores=None):
    from concourse.bass_utils import run_bass_kernel_spmd
    nc = _get_nc()
    return run_bass_kernel_spmd(
        nc, in_maps, core_ids=list(range(NCORES)),
        trace=trace, trace_cores=trace_cores,
    )


def kernel(features, weights, bias):
    in_maps = _prep_inputs(features, weights)
    res = _run(in_maps)
    return _gather(res.results, bias)
